# revision 1
# baseline (speedup 1.0000x reference)
"""Bass TRN2 kernel for nn_EtaWeights.

out[i] = loss[i]*mask*eta   if loss[i] > eta
       = -loss[i]/eta + 1   otherwise

Data-parallel over the single axis: 8 cores, each streams a contiguous
2^22-element shard of the 2^25-element vector through SBUF.

Fast path (mask*eta == 0, the shipped parameter values): the true-branch is
identically 0 and the false-branch 1 - x/eta crosses zero exactly at x = eta,
so out == -min(x - eta, 0) / eta exactly (fp32 rounding is symmetric under
negation, and the +/-0 difference on the clamped branch is value-equal).
Pipeline: SP issues in-DMAs (HWDGE), DVE runs one fused tensor_scalar
(subtract, min) in place, ACT scales by -1/eta (Copy activation) and issues
the out-DMA in program order. Raw Bass with explicit slot semaphores — Tile
would attach >1 sync-wait to DMA instructions, which walrus rejects.

General path (mask*eta != 0): all-DVE compare + predicated copy; ACT only
issues out-DMAs.
"""

import numpy as np

N = 33554432  # 2**25
NCORES = 8
PER_CORE = N // NCORES  # 2**22

P = 128  # SBUF partitions
NT = 8  # tiles per core
F = PER_CORE // (NT * P)  # 4096 -> 2 MiB per tile
BUFS = 6

TRACE = False
LAST_EXEC_NS = None
LAST_RESULTS = None

_module_cache = {}


def _build(e: float, m: float, nt: int = NT, f: int = F, repeats: int = 1,
           bufs: int = BUFS):
    from contextlib import ExitStack

    import concourse.bass as bass
    import concourse.mybir as mybir

    fp32 = mybir.dt.float32
    alu = mybir.AluOpType
    nc = bass.Bass("TRN2", target_bir_lowering=False, debug=False,
                   num_devices=NCORES)
    x = nc.dram_tensor("x", [nt, P, f], fp32, kind="ExternalInput").ap()
    y = nc.dram_tensor("y", [nt, P, f], fp32, kind="ExternalOutput").ap()

    total = nt * repeats
    fast = m * e == 0.0

    with ExitStack() as ctx:
        buf = ctx.enter_context(nc.sbuf_tensor([P, f * bufs], fp32))
        tiles = [buf[:, k * f:(k + 1) * f] for k in range(bufs)]
        if not fast:
            aux = ctx.enter_context(nc.sbuf_tensor([P, f], fp32))
            tr_t = aux[:, 0:f]
            # walrus requires an integer-dtype mask for CopyPredicated
            gt_buf = ctx.enter_context(
                nc.sbuf_tensor([P, f], mybir.dt.uint8)
            )
            gt_t = gt_buf[:, 0:f]
        block = ctx.enter_context(nc.Block(no_gpsimd_drain=True))
        in_sems = [nc.alloc_semaphore(f"in{k}") for k in range(bufs)]
        out_sems = [nc.alloc_semaphore(f"out{k}") for k in range(bufs)]
        dve_sem = nc.alloc_semaphore("dve")
        act_sem = nc.alloc_semaphore("act")
        uses = [len(range(k, total, bufs)) for k in range(bufs)]

        @block.sync
        def _(sp):
            for it in range(total):
                k, u = it % bufs, it // bufs
                if u > 0:
                    sp.wait_ge(out_sems[k], 16 * u)
                sp.dma_start(tiles[k], x[it % nt]).then_inc(in_sems[k], 16)
            for k in range(bufs):
                sp.wait_ge(out_sems[k], 16 * uses[k])

        @block.vector
        def _(dve):
            for it in range(total):
                k, u = it % bufs, it // bufs
                dve.wait_ge(in_sems[k], 16 * (u + 1))
                if fast:
                    # t = min(x - e, 0); ACT then scales by -1/e
                    dve.tensor_scalar(
                        tiles[k], tiles[k], e, 0.0, alu.subtract, alu.min
                    ).then_inc(dve_sem, 1)
                else:
                    # fully serialized on DVE (deep pipeline needs explicit
                    # sems even for same-engine dependencies); ACT waits for
                    # 5 chain ticks per iteration
                    ops = [
                        lambda: dve.tensor_scalar(gt_t, tiles[k], e, None,
                                                  alu.is_gt),
                        lambda: dve.tensor_scalar(tr_t, tiles[k], m * e,
                                                  None, alu.mult),
                        lambda: dve.tensor_scalar(tiles[k], tiles[k], e, 0.0,
                                                  alu.subtract, alu.min),
                        lambda: dve.tensor_scalar(tiles[k], tiles[k],
                                                  -1.0 / e, None, alu.mult),
                        lambda: dve.copy_predicated(tiles[k], gt_t, tr_t),
                    ]
                    for j, op in enumerate(ops):
                        dve.wait_ge(dve_sem, 5 * it + j)
                        op().then_inc(dve_sem, 1)

        @block.scalar
        def _(act):
            for it in range(total):
                k = it % bufs
                act.wait_ge(dve_sem, (it + 1) if fast else 5 * (it + 1))
                if fast:
                    # deep ACT pipeline: the HWDGE DMA issued by ACT does not
                    # implicitly wait for ACT's own in-flight compute
                    act.mul(tiles[k], tiles[k], -1.0 / e).then_inc(act_sem, 1)
                    act.wait_ge(act_sem, it + 1)
                act.dma_start(y[it % nt], tiles[k]).then_inc(out_sems[k], 16)

    return nc


def _build_phased(e: float, m: float, nt: int = NT, f: int = F,
                  repeats: int = 1):
    """Fast path (mask*eta == 0) with phased DMA: the whole 16.78 MiB shard
    fits in SBUF (128 KiB/partition), so read it all, compute on DVE, then
    write it all. Each direction alone saturates the ~435 GB/s SBUF fabric,
    while mixed-direction streaming tops out ~360 GB/s (HBM turnaround), so
    phasing beats the pipelined duplex schedule."""
    import concourse.bass as bass
    import concourse.mybir as mybir

    assert m * e == 0.0
    fp32 = mybir.dt.float32
    alu = mybir.AluOpType
    nc = bass.Bass("TRN2", target_bir_lowering=False, debug=False,
                   num_devices=NCORES)
    x = nc.dram_tensor("x", [nt, P, f], fp32, kind="ExternalInput").ap()
    y = nc.dram_tensor("y", [nt, P, f], fp32, kind="ExternalOutput").ap()

    with nc.sbuf_tensor([P, f * nt], fp32) as buf, \
            nc.Block(no_gpsimd_drain=True) as block:
        tiles = [buf[:, i * f:(i + 1) * f] for i in range(nt)]
        in_sems = [nc.alloc_semaphore(f"in{i}") for i in range(nt)]
        dve_sem = nc.alloc_semaphore("dve")
        out_sem = nc.alloc_semaphore("out")

        @block.sync
        def _(sp):
            for r in range(repeats):
                if r > 0:
                    # phase barrier: no reads while previous writes stream
                    sp.wait_ge(out_sem, 16 * nt * r)
                for i in range(nt):
                    sp.dma_start(tiles[i], x[i]).then_inc(in_sems[i], 16)
            sp.wait_ge(out_sem, 16 * nt * repeats)

        @block.vector
        def _(dve):
            for r in range(repeats):
                for i in range(nt):
                    it = nt * r + i
                    dve.wait_ge(in_sems[i], 16 * (r + 1))
                    dve.tensor_scalar(
                        tiles[i], tiles[i], e, 0.0, alu.subtract, alu.min
                    ).then_inc(dve_sem, 1)
                    dve.wait_ge(dve_sem, 2 * it + 1)
                    dve.tensor_scalar(
                        tiles[i], tiles[i], -1.0 / e, None, alu.mult
                    ).then_inc(dve_sem, 1)

        @block.scalar
        def _(act):
            for r in range(repeats):
                for j in range(nt):
                    act.wait_ge(in_sems[j], 16 * (r + 1))
                for i in range(nt):
                    act.wait_ge(dve_sem, 2 * (nt * r + i + 1))
                    act.dma_start(y[i], tiles[i]).then_inc(out_sem, 16)

    return nc


def _build_phased2(e: float, m: float, nt: int = NT, f: int = F,
                   repeats: int = 1):
    """Phased with the write phase split across both HWDGE rings (SP takes
    the first half of the tiles, ACT the second)."""
    import concourse.bass as bass
    import concourse.mybir as mybir

    assert m * e == 0.0
    fp32 = mybir.dt.float32
    alu = mybir.AluOpType
    nc = bass.Bass("TRN2", target_bir_lowering=False, debug=False,
                   num_devices=NCORES)
    x = nc.dram_tensor("x", [nt, P, f], fp32, kind="ExternalInput").ap()
    y = nc.dram_tensor("y", [nt, P, f], fp32, kind="ExternalOutput").ap()
    half = nt // 2

    with nc.sbuf_tensor([P, f * nt], fp32) as buf, \
            nc.Block(no_gpsimd_drain=True) as block:
        tiles = [buf[:, i * f:(i + 1) * f] for i in range(nt)]
        in_sems = [nc.alloc_semaphore(f"in{i}") for i in range(nt)]
        dve_sem = nc.alloc_semaphore("dve")
        out_sem = nc.alloc_semaphore("out")

        @block.sync
        def _(sp):
            for r in range(repeats):
                if r > 0:
                    sp.wait_ge(out_sem, 16 * nt * r)
                for i in range(nt):
                    sp.dma_start(tiles[i], x[i]).then_inc(in_sems[i], 16)
                for j in range(nt):
                    sp.wait_ge(in_sems[j], 16 * (r + 1))
                for i in range(half):
                    sp.wait_ge(dve_sem, 2 * (nt * r + i + 1))
                    sp.dma_start(y[i], tiles[i]).then_inc(out_sem, 16)
            sp.wait_ge(out_sem, 16 * nt * repeats)

        @block.vector
        def _(dve):
            for r in range(repeats):
                for i in range(nt):
                    it = nt * r + i
                    dve.wait_ge(in_sems[i], 16 * (r + 1))
                    dve.tensor_scalar(
                        tiles[i], tiles[i], e, 0.0, alu.subtract, alu.min
                    ).then_inc(dve_sem, 1)
                    dve.wait_ge(dve_sem, 2 * it + 1)
                    dve.tensor_scalar(
                        tiles[i], tiles[i], -1.0 / e, None, alu.mult
                    ).then_inc(dve_sem, 1)

        @block.scalar
        def _(act):
            for r in range(repeats):
                for j in range(nt):
                    act.wait_ge(in_sems[j], 16 * (r + 1))
                for i in range(half, nt):
                    act.wait_ge(dve_sem, 2 * (nt * r + i + 1))
                    act.dma_start(y[i], tiles[i]).then_inc(out_sem, 16)

    return nc


def _build_best(e: float, m: float, repeats: int = 1):
    if m * e == 0.0:
        return _build_phased2(e, m, repeats=repeats)
    return _build(e, m, repeats=repeats)


def kernel(loss: np.ndarray, eta: np.ndarray, mask: np.ndarray) -> np.ndarray:
    global LAST_EXEC_NS, LAST_RESULTS
    from concourse.bass_utils import run_bass_kernel_spmd

    loss = np.ascontiguousarray(np.asarray(loss, dtype=np.float32))
    e = float(np.asarray(eta).reshape(-1)[0])
    m = float(np.asarray(mask).reshape(-1)[0])
    assert loss.shape == (N,)

    key = (e, m)
    if key not in _module_cache:
        _module_cache[key] = _build_best(e, m)
    nc = _module_cache[key]

    shards = loss.reshape(NCORES, NT, P, F)
    in_maps = [{"x": shards[c]} for c in range(NCORES)]
    res = run_bass_kernel_spmd(
        nc, in_maps, core_ids=list(range(NCORES)), trace=TRACE
    )
    LAST_EXEC_NS = res.exec_time_ns
    LAST_RESULTS = res
    out = np.concatenate(
        [np.asarray(r["y"], dtype=np.float32).reshape(-1) for r in res.results]
    )
    return out



# revision 2
# speedup vs baseline: 1.1199x; 1.1199x over previous
"""Bass TRN2 kernel for nn_EtaWeights.

out[i] = loss[i]*mask*eta   if loss[i] > eta
       = -loss[i]/eta + 1   otherwise

Data-parallel over the single axis: 8 cores, each processing a contiguous
2^22-element shard viewed as [128 partitions x 32768] fp32 in SBUF-resident
tiles (16.78 MiB fits on-chip).

Fast path (mask*eta == 0 and eta > 0 — the shipped parameter values): both
branches collapse to out == relu(1 - loss/eta) exactly, which the ACT
(scalar) engine computes in ONE pass: activation(Relu, scale=-1/eta,
bias=1.0). Schedule is phased half-duplex (measured: the per-NC DMA path
sustains ~345 GB/s one direction but only ~320 GB/s mixed, so pure read
and write phases beat duplex streaming):

  SP ring:  5 read DMAs, uneven [4,4,4,3,1] MiB — the shrinking tail lets
            the last relu start early, cutting the read->write transition.
  ACT:      16 relu tiles (1 MiB) chasing the reads; then the write
            barrier (all relus done) and 16 write DMAs split across the
            SP and ACT HWDGE rings (split measurably beats one ring).

General path (mask*eta != 0 or eta <= 0): DVE compare + predicated copy
pipeline (correct for any eta/mask, slower; never hit by the shipped
inputs).
"""

import numpy as np

N = 33554432  # 2**25
NCORES = 8
PER_CORE = N // NCORES  # 2**22
P = 128
F_ALL = PER_CORE // P  # 32768 fp32 columns per partition

Q = 2048  # columns per MiB (2048 cols * 128 parts * 4 B = 1 MiB)
RBOUNDS = [0, 4 * Q, 8 * Q, 12 * Q, 15 * Q, 16 * Q]  # [4,4,4,3,1] MiB reads
NW = 16
FW = F_ALL // NW  # 2048 -> 1 MiB write/relu tiles

# general-path tiling (legacy layout)
NT = 8
F = PER_CORE // (NT * P)  # 4096

TRACE = False
LAST_EXEC_NS = None
LAST_RESULTS = None

_module_cache = {}


def _build_fast(e: float, repeats: int = 1):
    """Phased ACT-relu kernel; straight-line body for repeats == 1, hardware
    Fori loop otherwise (loop mode is for benchmarking only)."""
    import concourse.bass as bass
    import concourse.mybir as mybir

    assert e > 0.0
    fp32 = mybir.dt.float32
    act_fn = mybir.ActivationFunctionType
    nr = len(RBOUNDS) - 1

    nc = bass.Bass("TRN2", target_bir_lowering=False, debug=False,
                   num_devices=NCORES)
    x = nc.dram_tensor("x", [P, F_ALL], fp32, kind="ExternalInput").ap()
    y = nc.dram_tensor("y", [P, F_ALL], fp32, kind="ExternalOutput").ap()

    def rof(j):  # read tile containing write tile j
        c = j * FW
        for i in range(nr):
            if RBOUNDS[i] <= c < RBOUNDS[i + 1]:
                return i
        raise AssertionError

    sp_w = [j for j in range(NW) if j % 2 == 0]
    act_w = [j for j in range(NW) if j % 2 == 1]

    with nc.sbuf_tensor([P, F_ALL], fp32) as buf, \
            nc.Block(no_gpsimd_drain=True) as block:
        rt = [buf[:, RBOUNDS[i]:RBOUNDS[i + 1]] for i in range(nr)]
        wt = [buf[:, j * FW:(j + 1) * FW] for j in range(NW)]
        xr = [x[:, RBOUNDS[i]:RBOUNDS[i + 1]] for i in range(nr)]
        yw = [y[:, j * FW:(j + 1) * FW] for j in range(NW)]
        in_sems = [nc.alloc_semaphore(f"in{i}") for i in range(nr)]
        act_sem = nc.alloc_semaphore("act")
        out_sem = nc.alloc_semaphore("out")

        @block.sync
        def _(sp):
            def body(r1):
                for i in range(nr):
                    sp.dma_start(rt[i], xr[i]).then_inc(in_sems[i], 16)
                sp.wait_ge(act_sem, r1 * NW)
                for j in sp_w:
                    sp.dma_start(yw[j], wt[j]).then_inc(out_sem, 16)
                sp.wait_ge(out_sem, r1 * (16 * NW))

            if repeats == 1:
                body(1)
            else:
                with sp.Fori(1, repeats + 1) as r1:
                    body(r1)

        @block.scalar
        def _(act):
            def body(r1):
                t16 = r1 * 16
                for j in range(NW):
                    act.wait_ge(in_sems[rof(j)], t16)
                    act.activation(
                        wt[j], wt[j], act_fn.Relu, bias=1.0, scale=-1.0 / e,
                    ).then_inc(act_sem, 1)
                act.wait_ge(act_sem, r1 * NW)
                for j in act_w:
                    act.dma_start(yw[j], wt[j]).then_inc(out_sem, 16)
                act.wait_ge(out_sem, r1 * (16 * NW))

            if repeats == 1:
                body(1)
            else:
                with act.Fori(1, repeats + 1) as r1:
                    body(r1)

    return nc


def _build_general(e: float, m: float, nt: int = NT, f: int = F,
                   repeats: int = 1, bufs: int = 6):
    """Pipelined fallback for arbitrary (eta, mask): DVE compare + predicated
    copy; handles m*e != 0 and e < 0. Raw Bass with explicit slot sems."""
    from contextlib import ExitStack

    import concourse.bass as bass
    import concourse.mybir as mybir

    fp32 = mybir.dt.float32
    alu = mybir.AluOpType
    nc = bass.Bass("TRN2", target_bir_lowering=False, debug=False,
                   num_devices=NCORES)
    x = nc.dram_tensor("x", [nt, P, f], fp32, kind="ExternalInput").ap()
    y = nc.dram_tensor("y", [nt, P, f], fp32, kind="ExternalOutput").ap()

    total = nt * repeats
    fast = m * e == 0.0

    with ExitStack() as ctx:
        buf = ctx.enter_context(nc.sbuf_tensor([P, f * bufs], fp32))
        tiles = [buf[:, k * f:(k + 1) * f] for k in range(bufs)]
        if not fast:
            aux = ctx.enter_context(nc.sbuf_tensor([P, f], fp32))
            tr_t = aux[:, 0:f]
            # walrus requires an integer-dtype mask for CopyPredicated
            gt_buf = ctx.enter_context(
                nc.sbuf_tensor([P, f], mybir.dt.uint8)
            )
            gt_t = gt_buf[:, 0:f]
        block = ctx.enter_context(nc.Block(no_gpsimd_drain=True))
        in_sems = [nc.alloc_semaphore(f"in{k}") for k in range(bufs)]
        out_sems = [nc.alloc_semaphore(f"out{k}") for k in range(bufs)]
        dve_sem = nc.alloc_semaphore("dve")
        act_sem = nc.alloc_semaphore("act")
        uses = [len(range(k, total, bufs)) for k in range(bufs)]

        @block.sync
        def _(sp):
            for it in range(total):
                k, u = it % bufs, it // bufs
                if u > 0:
                    sp.wait_ge(out_sems[k], 16 * u)
                sp.dma_start(tiles[k], x[it % nt]).then_inc(in_sems[k], 16)
            for k in range(bufs):
                sp.wait_ge(out_sems[k], 16 * uses[k])

        @block.vector
        def _(dve):
            for it in range(total):
                k, u = it % bufs, it // bufs
                dve.wait_ge(in_sems[k], 16 * (u + 1))
                if fast:
                    # t = min(x - e, 0); ACT then scales by -1/e
                    dve.tensor_scalar(
                        tiles[k], tiles[k], e, 0.0, alu.subtract, alu.min
                    ).then_inc(dve_sem, 1)
                else:
                    # fully serialized on DVE (deep pipeline needs explicit
                    # sems even for same-engine dependencies); ACT waits for
                    # 5 chain ticks per iteration
                    ops = [
                        lambda: dve.tensor_scalar(gt_t, tiles[k], e, None,
                                                  alu.is_gt),
                        lambda: dve.tensor_scalar(tr_t, tiles[k], m * e,
                                                  None, alu.mult),
                        lambda: dve.tensor_scalar(tiles[k], tiles[k], e, 0.0,
                                                  alu.subtract, alu.min),
                        lambda: dve.tensor_scalar(tiles[k], tiles[k],
                                                  -1.0 / e, None, alu.mult),
                        lambda: dve.copy_predicated(tiles[k], gt_t, tr_t),
                    ]
                    for j, op in enumerate(ops):
                        dve.wait_ge(dve_sem, 5 * it + j)
                        op().then_inc(dve_sem, 1)

        @block.scalar
        def _(act):
            for it in range(total):
                k = it % bufs
                act.wait_ge(dve_sem, (it + 1) if fast else 5 * (it + 1))
                if fast:
                    # deep ACT pipeline: the HWDGE DMA issued by ACT does not
                    # implicitly wait for ACT's own in-flight compute
                    act.mul(tiles[k], tiles[k], -1.0 / e).then_inc(act_sem, 1)
                    act.wait_ge(act_sem, it + 1)
                act.dma_start(y[it % nt], tiles[k]).then_inc(out_sems[k], 16)

    return nc


def _build_best(e: float, m: float, repeats: int = 1):
    if m * e == 0.0 and e > 0.0:
        return _build_fast(e, repeats=repeats)
    return _build_general(e, m, repeats=repeats)


def _is_fast(e: float, m: float) -> bool:
    return m * e == 0.0 and e > 0.0


def kernel(loss: np.ndarray, eta: np.ndarray, mask: np.ndarray) -> np.ndarray:
    global LAST_EXEC_NS, LAST_RESULTS
    from concourse.bass_utils import run_bass_kernel_spmd

    loss = np.ascontiguousarray(np.asarray(loss, dtype=np.float32))
    e = float(np.asarray(eta).reshape(-1)[0])
    m = float(np.asarray(mask).reshape(-1)[0])
    assert loss.shape == (N,)

    key = (e, m)
    if key not in _module_cache:
        _module_cache[key] = _build_best(e, m)
    nc = _module_cache[key]

    if _is_fast(e, m):
        shards = loss.reshape(NCORES, P, F_ALL)
    else:
        shards = loss.reshape(NCORES, NT, P, F)
    in_maps = [{"x": shards[c]} for c in range(NCORES)]
    res = run_bass_kernel_spmd(
        nc, in_maps, core_ids=list(range(NCORES)), trace=TRACE
    )
    LAST_EXEC_NS = res.exec_time_ns
    LAST_RESULTS = res
    out = np.concatenate(
        [np.asarray(r["y"], dtype=np.float32).reshape(-1) for r in res.results]
    )
    return out
